# revision 1
# baseline (speedup 1.0000x reference)
"""Multi-head attention (B=4, T=2048, D=1024, H=16) on 8 TRN2 NeuronCores.

Sharding: core c handles batch b = c//2 and head-half hh = c%2 (8 heads,
512 of the 1024 channel dims). Each core computes its half of the head
outputs and a row-sharded output projection, producing a partial
[T, D] output. Host unshard: out[b] = partial[2b] + partial[2b+1]
+ b_o + b_v @ w_o.T (the value-bias contribution commutes through
attention because softmax rows sum to 1).

All matmuls run in float32r (hi/lo split on the PE at full rate,
rms rel err ~1.5e-4 per matmul).
"""

from contextlib import ExitStack

import numpy as np

import concourse.bass as bass
import concourse.mybir as mybir
import concourse.tile as tile
from concourse import bacc
from concourse.bass_utils import run_bass_kernel_spmd

B, T, D = 4, 2048, 1024
H = 16
DH = 64  # head dim
HALF = 512  # channels per core (8 heads)
N_CORES = 8

F32 = mybir.dt.float32
F32R = mybir.dt.float32r

TB = 512  # t-block for moving operands
NTB = T // TB  # 4
KB = 128  # contraction block
NKB = D // KB  # 8
NJB = HALF // KB  # 4 j-blocks of the half
NTK = T // KB  # 16 tk blocks
GRP = 2  # tk-blocks per exp group (4 psum banks: 2 heads x 2 tk)
NGRP = NTK // GRP  # 8


def r(ap):
    return ap.bitcast(F32R)


def build_kernel():
    nc = bacc.Bacc(
        "TRN2", target_bir_lowering=False, debug=False, num_devices=N_CORES
    )
    xqT = nc.dram_tensor("xqT", [D, T], F32R, kind="ExternalInput").ap()
    xkT = nc.dram_tensor("xkT", [D, T], F32R, kind="ExternalInput").ap()
    xvT = nc.dram_tensor("xvT", [D, T], F32R, kind="ExternalInput").ap()
    wqT = nc.dram_tensor("wqT", [D, HALF], F32R, kind="ExternalInput").ap()
    wkT = nc.dram_tensor("wkT", [D, HALF], F32R, kind="ExternalInput").ap()
    wvT = nc.dram_tensor("wvT", [D, HALF], F32R, kind="ExternalInput").ap()
    woT = nc.dram_tensor("woT", [HALF, D], F32R, kind="ExternalInput").ap()
    bq = nc.dram_tensor("bq", [HALF, 1], F32, kind="ExternalInput").ap()
    bk = nc.dram_tensor("bk", [HALF, 1], F32, kind="ExternalInput").ap()
    ones_in = nc.dram_tensor("ones_in", [KB, H // 2], F32R, kind="ExternalInput").ap()
    partial = nc.dram_tensor("partial", [T, D], F32, kind="ExternalOutput").ap()

    with tile.TileContext(nc) as tc, ExitStack() as ctx:
        p_const = ctx.enter_context(tc.tile_pool(name="const", bufs=1))
        p_kt = ctx.enter_context(tc.tile_pool(name="kt", bufs=NJB))
        p_v = ctx.enter_context(tc.tile_pool(name="v", bufs=NTK))
        p_qt = ctx.enter_context(tc.tile_pool(name="qt", bufs=2 * NJB))
        p_xs = ctx.enter_context(tc.tile_pool(name="xs", bufs=3))
        p_ex = ctx.enter_context(tc.tile_pool(name="ex", bufs=2))
        p_ot = ctx.enter_context(tc.tile_pool(name="ot", bufs=2 * NJB))
        p_rc = ctx.enter_context(tc.tile_pool(name="rc", bufs=2))
        p_st = ctx.enter_context(tc.tile_pool(name="st", bufs=2))
        # PSUM: scores 4 banks + av 2 + proj/outproj 2 = 8
        p_sc = ctx.enter_context(tc.tile_pool(name="sc", bufs=1, space="PSUM"))
        p_av = ctx.enter_context(tc.tile_pool(name="av", bufs=2, space="PSUM"))
        p_po = ctx.enter_context(tc.tile_pool(name="po", bufs=2, space="PSUM"))

        # ---- constants ----
        w_q = p_const.tile([KB, NKB, HALF], F32R, tag="wq")
        nc.sync.dma_start(w_q[:], wqT.rearrange("(kb p) j -> p kb j", p=KB))
        w_k = p_const.tile([KB, NKB, HALF], F32R, tag="wk")
        nc.sync.dma_start(w_k[:], wkT.rearrange("(kb p) j -> p kb j", p=KB))
        w_v = p_const.tile([KB, NKB, HALF], F32R, tag="wv")
        nc.sync.dma_start(w_v[:], wvT.rearrange("(kb p) j -> p kb j", p=KB))
        w_o = p_const.tile([KB, NJB, D], F32R, tag="wo")
        nc.sync.dma_start(w_o[:], woT.rearrange("(jb p) n -> p jb n", p=KB))
        b_q = p_const.tile([KB, NJB], F32, tag="bq")
        nc.sync.dma_start(b_q[:], bq.rearrange("(jb p) one -> p (jb one)", p=KB))
        b_k = p_const.tile([KB, NJB], F32, tag="bk")
        nc.sync.dma_start(b_k[:], bk.rearrange("(jb p) one -> p (jb one)", p=KB))
        ones8 = p_const.tile([KB, H // 2], F32R, tag="ones8")
        nc.sync.dma_start(ones8[:], ones_in[:])

        # ---- K^T projection: KT[jb] is [128 (j), T] ----
        kt_tiles = [p_kt.tile([KB, T], F32R, tag="kt", name=f"kt{j}") for j in range(NJB)]
        for tb in range(NTB):
            ps = p_sc.tile([KB, 4 * TB], F32, tag="sc")
            for kb in range(NKB):
                xt = p_xs.tile([KB, TB], F32R, tag="xs")
                nc.sync.dma_start(xt[:], xkT[kb * KB : (kb + 1) * KB, tb * TB : (tb + 1) * TB])
                for jb in range(NJB):
                    nc.tensor.matmul(
                        ps[:, jb * TB : (jb + 1) * TB],
                        r(w_k[:, kb, jb * KB : (jb + 1) * KB]),
                        r(xt[:]),
                        start=(kb == 0),
                        stop=(kb == NKB - 1),
                    )
            for jb in range(NJB):
                nc.vector.tensor_scalar_add(
                    kt_tiles[jb][:, tb * TB : (tb + 1) * TB],
                    ps[:, jb * TB : (jb + 1) * TB],
                    b_k[:, jb : jb + 1],
                )

        # ---- V projection (natural layout): V[tk] is [128 (t), HALF (j)] ----
        v_tiles = [
            p_v.tile([KB, H // 2, DH + 1], F32R, tag="v", name=f"v{j}")
            for j in range(NTK)
        ]
        for t in range(NTK):
            nc.sync.dma_start(v_tiles[t][:, :, DH : DH + 1], ones8[:, :, None])
        for tb in range(NTB):
            ps = p_sc.tile([KB, 4 * TB], F32, tag="sc")
            for kb in range(NKB):
                xt = p_xs.tile([KB, TB], F32R, tag="xs")
                nc.sync.dma_start(xt[:], xvT[kb * KB : (kb + 1) * KB, tb * TB : (tb + 1) * TB])
                for ts in range(4):
                    nc.tensor.matmul(
                        ps[:, ts * TB : (ts + 1) * TB],
                        r(xt[:, ts * KB : (ts + 1) * KB]),
                        r(w_v[:, kb, :]),
                        start=(kb == 0),
                        stop=(kb == NKB - 1),
                    )
            for ts in range(4):
                nc.vector.tensor_copy(
                    v_tiles[tb * 4 + ts][:, :, 0:DH],
                    ps[:, ts * TB : (ts + 1) * TB].rearrange("p (h d) -> p h d", d=DH),
                )

        # ---- per t-block: Q^T projection, attention, out-projection ----
        for tq in range(NTB):
            # Q^T for this t-block: qt[jb] [128 (j), TB]
            qt_tiles = [p_qt.tile([KB, TB], F32R, tag="qt", name=f"qt{j}") for j in range(NJB)]
            ps = p_sc.tile([KB, 4 * TB], F32, tag="sc")
            for kb in range(NKB):
                xt = p_xs.tile([KB, TB], F32R, tag="xs")
                nc.sync.dma_start(xt[:], xqT[kb * KB : (kb + 1) * KB, tq * TB : (tq + 1) * TB])
                for jb in range(NJB):
                    nc.tensor.matmul(
                        ps[:, jb * TB : (jb + 1) * TB],
                        r(w_q[:, kb, jb * KB : (jb + 1) * KB]),
                        r(xt[:]),
                        start=(kb == 0),
                        stop=(kb == NKB - 1),
                    )
            for jb in range(NJB):
                nc.vector.tensor_scalar_add(
                    qt_tiles[jb][:], ps[:, jb * TB : (jb + 1) * TB], b_q[:, jb : jb + 1]
                )

            ot_tiles = [p_ot.tile([KB, TB], F32R, tag="ot", name=f"ot{j}") for j in range(NJB)]
            for jp in range(NJB):  # head pair (2*jp, 2*jp+1)
                # fp32r matmuls require dst partition base 0, so each head of
                # the pair accumulates into its own psum tile; row DH carries the
                # softmax denominator via the ones column interleaved in V.
                avs = [p_av.tile([DH + 1, TB], F32, tag="av", name=f"av{i}") for i in range(2)]
                for g in range(NGRP):
                    sc = p_sc.tile([KB, 4 * TB], F32, tag="sc")
                    # scores: S^T[tk-block, tq] for both heads of the pair
                    for i in range(2):
                        for u in range(GRP):
                            tk = g * GRP + u
                            slot = i * GRP + u
                            nc.tensor.matmul(
                                sc[:, slot * TB : (slot + 1) * TB],
                                r(kt_tiles[jp][i * DH : (i + 1) * DH, tk * KB : (tk + 1) * KB]),
                                r(qt_tiles[jp][i * DH : (i + 1) * DH, :]),
                                start=True,
                                stop=True,
                            )
                    ex = p_ex.tile([KB, 4 * TB], F32R, tag="ex")
                    nc.scalar.activation(
                        ex[:], sc[:], mybir.ActivationFunctionType.Exp, scale=0.125
                    )
                    for i in range(2):
                        for u in range(GRP):
                            tk = g * GRP + u
                            slot = i * GRP + u
                            exs = ex[:, slot * TB : (slot + 1) * TB]
                            nc.tensor.matmul(
                                avs[i][:],
                                r(v_tiles[tk][:, 2 * jp + i, :]),
                                r(exs),
                                start=(tk == 0),
                                stop=(tk == NTK - 1),
                            )
                for i in range(2):
                    # denominator row -> sbuf, DMA-broadcast across the 64
                    # head-dim partitions, reciprocal at base 0 (custom-DVE
                    # ops misbehave on nonzero partition base), then scale
                    dsb = p_rc.tile([DH + 1, TB], F32, tag="dsb")
                    nc.vector.tensor_copy(dsb[DH : DH + 1, :], avs[i][DH : DH + 1, :])
                    bc = p_rc.tile([DH, TB], F32, tag="bc")
                    nc.sync.dma_start(
                        bc[:],
                        dsb[DH : DH + 1, None, :].broadcast_to([1, DH, TB]),
                    )
                    rc2 = p_rc.tile([DH, TB], F32, tag="rc2")
                    nc.vector.reciprocal_approx_fast(rc2[:], bc[:])
                    if i == 0:
                        nc.vector.tensor_mul(ot_tiles[jp][0:DH, :], avs[i][0:DH, :], rc2[:])
                    else:
                        # DVE can't shift partitions; stage then DMA into rows 64:128
                        stg = p_rc.tile([DH, TB], F32R, tag="stg")
                        nc.vector.tensor_mul(stg[:], avs[i][0:DH, :], rc2[:])
                        nc.sync.dma_start(ot_tiles[jp][DH : 2 * DH, :], stg[:])

            # out-projection for this t-block
            for nb in range(2):
                for ts in range(4):
                    po = p_po.tile([KB, TB], F32, tag="po")
                    for jp in range(NJB):
                        nc.tensor.matmul(
                            po[:],
                            r(ot_tiles[jp][:, ts * KB : (ts + 1) * KB]),
                            r(w_o[:, jp, nb * TB : (nb + 1) * TB]),
                            start=(jp == 0),
                            stop=(jp == NJB - 1),
                        )
                    st = p_st.tile([KB, TB], F32, tag="st")
                    nc.vector.tensor_copy(st[:], po[:])
                    nc.sync.dma_start(
                        partial[
                            tq * TB + ts * KB : tq * TB + (ts + 1) * KB,
                            nb * TB : (nb + 1) * TB,
                        ],
                        st[:],
                    )

    nc.compile()
    return nc


def kernel(**inputs: np.ndarray) -> np.ndarray:
    query = np.asarray(inputs["query"], dtype=np.float32)
    key = np.asarray(inputs["key"], dtype=np.float32)
    value = np.asarray(inputs["value"], dtype=np.float32)
    w_q = np.asarray(inputs["w_q"], dtype=np.float32)
    b_q = np.asarray(inputs["b_q"], dtype=np.float32)
    w_k = np.asarray(inputs["w_k"], dtype=np.float32)
    b_k = np.asarray(inputs["b_k"], dtype=np.float32)
    w_v = np.asarray(inputs["w_v"], dtype=np.float32)
    b_v = np.asarray(inputs["b_v"], dtype=np.float32)
    w_o = np.asarray(inputs["w_o"], dtype=np.float32)
    b_o = np.asarray(inputs["b_o"], dtype=np.float32)

    nc = build_kernel()

    in_maps = []
    for c in range(N_CORES):
        b = c // 2
        hh = c % 2
        sl = slice(hh * HALF, (hh + 1) * HALF)
        in_maps.append(
            {
                "xqT": np.ascontiguousarray(query[b].T),
                "xkT": np.ascontiguousarray(key[b].T),
                "xvT": np.ascontiguousarray(value[b].T),
                "wqT": np.ascontiguousarray(w_q[sl, :].T),
                "wkT": np.ascontiguousarray(w_k[sl, :].T),
                "wvT": np.ascontiguousarray(w_v[sl, :].T),
                "woT": np.ascontiguousarray(w_o[:, sl].T),
                "bq": np.ascontiguousarray(b_q[sl].reshape(HALF, 1)),
                "bk": np.ascontiguousarray(b_k[sl].reshape(HALF, 1)),
                "ones_in": np.ones((KB, H // 2), dtype=np.float32),
            }
        )

    res = run_bass_kernel_spmd(nc, in_maps, core_ids=list(range(N_CORES)))

    const_row = (b_v[None, :] @ w_o.T + b_o[None, :]).astype(np.float32)
    out = np.empty((B, T, D), dtype=np.float32)
    for b in range(B):
        out[b] = res.results[2 * b]["partial"] + res.results[2 * b + 1]["partial"]
        out[b] += const_row
    return out



# revision 2
# speedup vs baseline: 1.1051x; 1.1051x over previous
"""Multi-head attention (B=4, T=2048, D=1024, H=16) on 8 TRN2 NeuronCores.

Sharding: core c handles batch b = c//2 and head-half hh = c%2 (8 heads,
512 of the 1024 channel dims). Each core computes its half of the head
outputs and a row-sharded output projection, producing a partial
[T, D] output. Host unshard: out[b] = partial[2b] + partial[2b+1]
+ b_o + b_v @ w_o.T (the value-bias contribution commutes through
attention because softmax rows sum to 1).

v2: all matmuls in bf16 (fp32r costs ~3 PE passes/row + 2x LDWEIGHTS;
bf16 is 1 pass). Host converts inputs/weights to bf16; on-device
psum->sbuf moves fuse the bf16 cast. Score matmuls issue i-inner so the
two 64-row head tiles of a pair run concurrently (row tiling).
"""

from contextlib import ExitStack

import numpy as np
import ml_dtypes

import concourse.bass as bass
import concourse.mybir as mybir
import concourse.tile as tile
from concourse import bacc
from concourse.bass_utils import run_bass_kernel_spmd

B, T, D = 4, 2048, 1024
H = 16
DH = 64  # head dim
HALF = 512  # channels per core (8 heads)
N_CORES = 8

F32 = mybir.dt.float32
BF16 = mybir.dt.bfloat16

TB = 512  # t-block for moving operands
NTB = T // TB  # 4
KB = 128  # contraction block
NKB = D // KB  # 8
NJB = HALF // KB  # 4 j-blocks of the half
NTK = T // KB  # 16 tk blocks
GRP = 2  # tk-blocks per exp group (4 psum banks: 2 heads x 2 tk)
NGRP = NTK // GRP  # 8


def build_kernel():
    nc = bacc.Bacc(
        "TRN2", target_bir_lowering=False, debug=False, num_devices=N_CORES
    )
    xqT = nc.dram_tensor("xqT", [D, T], BF16, kind="ExternalInput").ap()
    xkT = nc.dram_tensor("xkT", [D, T], BF16, kind="ExternalInput").ap()
    xvT = nc.dram_tensor("xvT", [D, T], BF16, kind="ExternalInput").ap()
    wqT = nc.dram_tensor("wqT", [D, HALF], BF16, kind="ExternalInput").ap()
    wkT = nc.dram_tensor("wkT", [D, HALF], BF16, kind="ExternalInput").ap()
    wvT = nc.dram_tensor("wvT", [D, HALF], BF16, kind="ExternalInput").ap()
    woT = nc.dram_tensor("woT", [HALF, D], BF16, kind="ExternalInput").ap()
    bq = nc.dram_tensor("bq", [HALF, 1], F32, kind="ExternalInput").ap()
    bk = nc.dram_tensor("bk", [HALF, 1], F32, kind="ExternalInput").ap()
    ones_in = nc.dram_tensor("ones_in", [KB, H // 2], BF16, kind="ExternalInput").ap()
    partial = nc.dram_tensor("partial", [T, D], F32, kind="ExternalOutput").ap()

    with tile.TileContext(nc) as tc, ExitStack() as ctx:
        p_const = ctx.enter_context(tc.tile_pool(name="const", bufs=1))
        p_kt = ctx.enter_context(tc.tile_pool(name="kt", bufs=NJB))
        p_v = ctx.enter_context(tc.tile_pool(name="v", bufs=NTK))
        p_qt = ctx.enter_context(tc.tile_pool(name="qt", bufs=2 * NJB))
        p_xs = ctx.enter_context(tc.tile_pool(name="xs", bufs=3))
        p_ex = ctx.enter_context(tc.tile_pool(name="ex", bufs=3))
        p_ot = ctx.enter_context(tc.tile_pool(name="ot", bufs=2 * NJB))
        p_rc = ctx.enter_context(tc.tile_pool(name="rc", bufs=2))
        p_st = ctx.enter_context(tc.tile_pool(name="st", bufs=2))
        # PSUM: scores 4 banks + av 2 + proj/outproj 2 = 8
        p_sc = ctx.enter_context(tc.tile_pool(name="sc", bufs=1, space="PSUM"))
        p_av = ctx.enter_context(tc.tile_pool(name="av", bufs=2, space="PSUM"))
        p_po = ctx.enter_context(tc.tile_pool(name="po", bufs=2, space="PSUM"))

        # ---- constants ----
        w_q = p_const.tile([KB, NKB, HALF], BF16, tag="wq")
        nc.sync.dma_start(w_q[:], wqT.rearrange("(kb p) j -> p kb j", p=KB))
        w_k = p_const.tile([KB, NKB, HALF], BF16, tag="wk")
        nc.sync.dma_start(w_k[:], wkT.rearrange("(kb p) j -> p kb j", p=KB))
        w_v = p_const.tile([KB, NKB, HALF], BF16, tag="wv")
        nc.sync.dma_start(w_v[:], wvT.rearrange("(kb p) j -> p kb j", p=KB))
        w_o = p_const.tile([KB, NJB, D], BF16, tag="wo")
        nc.sync.dma_start(w_o[:], woT.rearrange("(jb p) n -> p jb n", p=KB))
        b_q = p_const.tile([KB, NJB], F32, tag="bq")
        nc.sync.dma_start(b_q[:], bq.rearrange("(jb p) one -> p (jb one)", p=KB))
        b_k = p_const.tile([KB, NJB], F32, tag="bk")
        nc.sync.dma_start(b_k[:], bk.rearrange("(jb p) one -> p (jb one)", p=KB))
        ones8 = p_const.tile([KB, H // 2], BF16, tag="ones8")
        nc.sync.dma_start(ones8[:], ones_in[:])

        # ---- K^T projection: KT[jb] is [128 (j), T] ----
        kt_tiles = [p_kt.tile([KB, T], BF16, tag="kt", name=f"kt{j}") for j in range(NJB)]
        for tb in range(NTB):
            ps = p_sc.tile([KB, 4 * TB], F32, tag="sc")
            for kb in range(NKB):
                xt = p_xs.tile([KB, TB], BF16, tag="xs")
                nc.sync.dma_start(xt[:], xkT[kb * KB : (kb + 1) * KB, tb * TB : (tb + 1) * TB])
                for jb in range(NJB):
                    nc.tensor.matmul(
                        ps[:, jb * TB : (jb + 1) * TB],
                        w_k[:, kb, jb * KB : (jb + 1) * KB],
                        xt[:],
                        start=(kb == 0),
                        stop=(kb == NKB - 1),
                    )
            for jb in range(NJB):
                nc.vector.tensor_scalar_add(
                    kt_tiles[jb][:, tb * TB : (tb + 1) * TB],
                    ps[:, jb * TB : (jb + 1) * TB],
                    b_k[:, jb : jb + 1],
                )

        # ---- V projection (natural layout): V[tk] is [128 (t), HALF (j)] ----
        v_tiles = [
            p_v.tile([KB, H // 2, DH + 1], BF16, tag="v", name=f"v{j}")
            for j in range(NTK)
        ]
        for t in range(NTK):
            nc.sync.dma_start(v_tiles[t][:, :, DH : DH + 1], ones8[:, :, None])
        for tb in range(NTB):
            ps = p_sc.tile([KB, 4 * TB], F32, tag="sc")
            for kb in range(NKB):
                xt = p_xs.tile([KB, TB], BF16, tag="xs")
                nc.sync.dma_start(xt[:], xvT[kb * KB : (kb + 1) * KB, tb * TB : (tb + 1) * TB])
                for ts in range(4):
                    nc.tensor.matmul(
                        ps[:, ts * TB : (ts + 1) * TB],
                        xt[:, ts * KB : (ts + 1) * KB],
                        w_v[:, kb, :],
                        start=(kb == 0),
                        stop=(kb == NKB - 1),
                    )
            for ts in range(4):
                nc.vector.tensor_copy(
                    v_tiles[tb * 4 + ts][:, :, 0:DH],
                    ps[:, ts * TB : (ts + 1) * TB].rearrange("p (h d) -> p h d", d=DH),
                )

        # ---- per t-block: Q^T projection, attention, out-projection ----
        for tq in range(NTB):
            # Q^T for this t-block: qt[jb] [128 (j), TB]
            qt_tiles = [p_qt.tile([KB, TB], BF16, tag="qt", name=f"qt{j}") for j in range(NJB)]
            ps = p_sc.tile([KB, 4 * TB], F32, tag="sc")
            for kb in range(NKB):
                xt = p_xs.tile([KB, TB], BF16, tag="xs")
                nc.sync.dma_start(xt[:], xqT[kb * KB : (kb + 1) * KB, tq * TB : (tq + 1) * TB])
                for jb in range(NJB):
                    nc.tensor.matmul(
                        ps[:, jb * TB : (jb + 1) * TB],
                        w_q[:, kb, jb * KB : (jb + 1) * KB],
                        xt[:],
                        start=(kb == 0),
                        stop=(kb == NKB - 1),
                    )
            for jb in range(NJB):
                nc.vector.tensor_scalar_add(
                    qt_tiles[jb][:], ps[:, jb * TB : (jb + 1) * TB], b_q[:, jb : jb + 1]
                )

            ot_tiles = [p_ot.tile([KB, TB], BF16, tag="ot", name=f"ot{j}") for j in range(NJB)]
            for jp in range(NJB):  # head pair (2*jp, 2*jp+1)
                # each head of the pair accumulates into its own psum tile;
                # row DH carries the softmax denominator via the ones column
                # interleaved in V.
                avs = [p_av.tile([DH + 1, TB], F32, tag="av", name=f"av{i}") for i in range(2)]
                for g in range(NGRP):
                    sc = p_sc.tile([KB, 4 * TB], F32, tag="sc")
                    # scores: S^T[tk-block, tq] for both heads of the pair.
                    # i inner: consecutive matmuls hit row groups 0 and 64,
                    # so the pair runs concurrently in the PE array.
                    for u in range(GRP):
                        tk = g * GRP + u
                        for i in range(2):
                            slot = i * GRP + u
                            nc.tensor.matmul(
                                sc[:, slot * TB : (slot + 1) * TB],
                                kt_tiles[jp][i * DH : (i + 1) * DH, tk * KB : (tk + 1) * KB],
                                qt_tiles[jp][i * DH : (i + 1) * DH, :],
                                start=True,
                                stop=True,
                            )
                    ex = p_ex.tile([KB, 4 * TB], BF16, tag="ex")
                    nc.scalar.activation(
                        ex[:], sc[:], mybir.ActivationFunctionType.Exp, scale=0.125
                    )
                    for u in range(GRP):
                        tk = g * GRP + u
                        for i in range(2):
                            slot = i * GRP + u
                            exs = ex[:, slot * TB : (slot + 1) * TB]
                            nc.tensor.matmul(
                                avs[i][:],
                                v_tiles[tk][:, 2 * jp + i, :],
                                exs,
                                start=(tk == 0),
                                stop=(tk == NTK - 1),
                            )
                for i in range(2):
                    # denominator row -> sbuf, DMA-broadcast across the 64
                    # head-dim partitions, reciprocal at base 0 (custom-DVE
                    # ops misbehave on nonzero partition base), then scale
                    dsb = p_rc.tile([DH + 1, TB], F32, tag="dsb")
                    nc.vector.tensor_copy(dsb[DH : DH + 1, :], avs[i][DH : DH + 1, :])
                    bc = p_rc.tile([DH, TB], F32, tag="bc")
                    nc.sync.dma_start(
                        bc[:],
                        dsb[DH : DH + 1, None, :].broadcast_to([1, DH, TB]),
                    )
                    rc2 = p_rc.tile([DH, TB], F32, tag="rc2")
                    nc.vector.reciprocal_approx_fast(rc2[:], bc[:])
                    if i == 0:
                        nc.vector.tensor_mul(ot_tiles[jp][0:DH, :], avs[i][0:DH, :], rc2[:])
                    else:
                        # DVE can't shift partitions; stage then DMA into rows 64:128
                        stg = p_rc.tile([DH, TB], BF16, tag="stg")
                        nc.vector.tensor_mul(stg[:], avs[i][0:DH, :], rc2[:])
                        nc.sync.dma_start(ot_tiles[jp][DH : 2 * DH, :], stg[:])

            # out-projection for this t-block
            for nb in range(2):
                for ts in range(4):
                    po = p_po.tile([KB, TB], F32, tag="po")
                    for jp in range(NJB):
                        nc.tensor.matmul(
                            po[:],
                            ot_tiles[jp][:, ts * KB : (ts + 1) * KB],
                            w_o[:, jp, nb * TB : (nb + 1) * TB],
                            start=(jp == 0),
                            stop=(jp == NJB - 1),
                        )
                    st = p_st.tile([KB, TB], F32, tag="st")
                    nc.vector.tensor_copy(st[:], po[:])
                    nc.sync.dma_start(
                        partial[
                            tq * TB + ts * KB : tq * TB + (ts + 1) * KB,
                            nb * TB : (nb + 1) * TB,
                        ],
                        st[:],
                    )

    nc.compile()
    return nc


def kernel(**inputs: np.ndarray) -> np.ndarray:
    query = np.asarray(inputs["query"], dtype=np.float32)
    key = np.asarray(inputs["key"], dtype=np.float32)
    value = np.asarray(inputs["value"], dtype=np.float32)
    w_q = np.asarray(inputs["w_q"], dtype=np.float32)
    b_q = np.asarray(inputs["b_q"], dtype=np.float32)
    w_k = np.asarray(inputs["w_k"], dtype=np.float32)
    b_k = np.asarray(inputs["b_k"], dtype=np.float32)
    w_v = np.asarray(inputs["w_v"], dtype=np.float32)
    b_v = np.asarray(inputs["b_v"], dtype=np.float32)
    w_o = np.asarray(inputs["w_o"], dtype=np.float32)
    b_o = np.asarray(inputs["b_o"], dtype=np.float32)

    nc = build_kernel()

    bf = ml_dtypes.bfloat16
    in_maps = []
    for c in range(N_CORES):
        b = c // 2
        hh = c % 2
        sl = slice(hh * HALF, (hh + 1) * HALF)
        in_maps.append(
            {
                "xqT": np.ascontiguousarray(query[b].T.astype(bf)),
                "xkT": np.ascontiguousarray(key[b].T.astype(bf)),
                "xvT": np.ascontiguousarray(value[b].T.astype(bf)),
                "wqT": np.ascontiguousarray(w_q[sl, :].T.astype(bf)),
                "wkT": np.ascontiguousarray(w_k[sl, :].T.astype(bf)),
                "wvT": np.ascontiguousarray(w_v[sl, :].T.astype(bf)),
                "woT": np.ascontiguousarray(w_o[:, sl].T.astype(bf)),
                "bq": np.ascontiguousarray(b_q[sl].reshape(HALF, 1)),
                "bk": np.ascontiguousarray(b_k[sl].reshape(HALF, 1)),
                "ones_in": np.ones((KB, H // 2), dtype=bf),
            }
        )

    res = run_bass_kernel_spmd(nc, in_maps, core_ids=list(range(N_CORES)))

    const_row = (b_v[None, :] @ w_o.T + b_o[None, :]).astype(np.float32)
    out = np.empty((B, T, D), dtype=np.float32)
    for b in range(B):
        out[b] = res.results[2 * b]["partial"] + res.results[2 * b + 1]["partial"]
        out[b] += const_row
    return out


# revision 3
# speedup vs baseline: 1.4251x; 1.2895x over previous
"""Multi-head attention (B=4, T=2048, D=1024, H=16) on 8 TRN2 NeuronCores.

Sharding: core c handles batch b = c//2 and head-half hh = c%2 (8 heads,
512 of the 1024 channel dims). Each core computes its half of the head
outputs and a row-sharded output projection, producing a partial
[T, D] output. Host unshard: out[b] = partial[2b] + partial[2b+1]
+ b_o + b_v @ w_o.T (the value-bias contribution commutes through
attention because softmax rows sum to 1).

v3: all-bf16 matmuls. PSUM layout keeps three independent streams:
  - scores: 2 bufs x [128, 1024] (4 banks), double-buffered so the
    next score pair never waits on the exp of the previous group
  - av: 2 bufs x [65, 512] (2 banks), per-head attention-output
    accumulators (row 64 = softmax denominator via ones column in V)
  - pj: 2 bufs x [128, 512] (2 banks), shared by QKV projections and
    the output projection - single-bank accumulation chains
The projection chains are the PE's filler work during exp latency, so
the PE never idles long enough for the HAM clock gate to re-throttle.
"""

from contextlib import ExitStack

import numpy as np
import ml_dtypes

import concourse.bass as bass
import concourse.mybir as mybir
import concourse.tile as tile
from concourse import bacc
from concourse.bass_utils import run_bass_kernel_spmd

B, T, D = 4, 2048, 1024
H = 16
DH = 64  # head dim
HALF = 512  # channels per core (8 heads)
N_CORES = 8

F32 = mybir.dt.float32
BF16 = mybir.dt.bfloat16

TB = 512  # t-block for moving operands
NTB = T // TB  # 4
KB = 128  # contraction block
NKB = D // KB  # 8
NJB = HALF // KB  # 4 j-blocks of the half
NTK = T // KB  # 16 tk blocks


def build_kernel():
    nc = bacc.Bacc(
        "TRN2", target_bir_lowering=False, debug=False, num_devices=N_CORES
    )
    xqT = nc.dram_tensor("xqT", [D, T], BF16, kind="ExternalInput").ap()
    xkT = nc.dram_tensor("xkT", [D, T], BF16, kind="ExternalInput").ap()
    xvT = nc.dram_tensor("xvT", [D, T], BF16, kind="ExternalInput").ap()
    wqT = nc.dram_tensor("wqT", [D, HALF], BF16, kind="ExternalInput").ap()
    wkT = nc.dram_tensor("wkT", [D, HALF], BF16, kind="ExternalInput").ap()
    wvT = nc.dram_tensor("wvT", [D, HALF], BF16, kind="ExternalInput").ap()
    woT = nc.dram_tensor("woT", [HALF, D], BF16, kind="ExternalInput").ap()
    bq = nc.dram_tensor("bq", [HALF, 1], F32, kind="ExternalInput").ap()
    bk = nc.dram_tensor("bk", [HALF, 1], F32, kind="ExternalInput").ap()
    ones_in = nc.dram_tensor("ones_in", [KB, H // 2], BF16, kind="ExternalInput").ap()
    partial = nc.dram_tensor("partial", [T, D], F32, kind="ExternalOutput").ap()

    with tile.TileContext(nc) as tc, ExitStack() as ctx:
        p_const = ctx.enter_context(tc.tile_pool(name="const", bufs=1))
        p_kt = ctx.enter_context(tc.tile_pool(name="kt", bufs=NJB))
        p_v = ctx.enter_context(tc.tile_pool(name="v", bufs=NTK))
        p_qt = ctx.enter_context(tc.tile_pool(name="qt", bufs=2 * NJB))
        p_xs = ctx.enter_context(tc.tile_pool(name="xs", bufs=10))
        p_ex = ctx.enter_context(tc.tile_pool(name="ex", bufs=3))
        p_ot = ctx.enter_context(tc.tile_pool(name="ot", bufs=2 * NJB))
        p_rc = ctx.enter_context(tc.tile_pool(name="rc", bufs=2))
        p_st = ctx.enter_context(tc.tile_pool(name="st", bufs=2))
        # PSUM: scores 2x2 banks + av 2x1 + proj/outproj 2x1 = 8 banks
        p_sc = ctx.enter_context(tc.tile_pool(name="sc", bufs=2, space="PSUM"))
        p_av = ctx.enter_context(tc.tile_pool(name="av", bufs=2, space="PSUM"))
        p_pj = ctx.enter_context(tc.tile_pool(name="pj", bufs=2, space="PSUM"))

        # ---- constants ----
        w_q = p_const.tile([KB, NKB, HALF], BF16, tag="wq")
        nc.sync.dma_start(w_q[:], wqT.rearrange("(kb p) j -> p kb j", p=KB))
        w_k = p_const.tile([KB, NKB, HALF], BF16, tag="wk")
        nc.sync.dma_start(w_k[:], wkT.rearrange("(kb p) j -> p kb j", p=KB))
        w_v = p_const.tile([KB, NKB, HALF], BF16, tag="wv")
        nc.sync.dma_start(w_v[:], wvT.rearrange("(kb p) j -> p kb j", p=KB))
        w_o = p_const.tile([KB, NJB, D], BF16, tag="wo")
        nc.sync.dma_start(w_o[:], woT.rearrange("(jb p) n -> p jb n", p=KB))
        b_q = p_const.tile([KB, NJB], F32, tag="bq")
        nc.sync.dma_start(b_q[:], bq.rearrange("(jb p) one -> p (jb one)", p=KB))
        b_k = p_const.tile([KB, NJB], F32, tag="bk")
        nc.sync.dma_start(b_k[:], bk.rearrange("(jb p) one -> p (jb one)", p=KB))
        ones8 = p_const.tile([KB, H // 2], BF16, tag="ones8")
        nc.sync.dma_start(ones8[:], ones_in[:])

        def load_x_tiles(src, tb):
            """DMA one t-block of an input into 8 resident [128, 512] tiles."""
            xts = []
            for kb in range(NKB):
                xt = p_xs.tile([KB, TB], BF16, tag="xs")
                nc.sync.dma_start(
                    xt[:], src[kb * KB : (kb + 1) * KB, tb * TB : (tb + 1) * TB]
                )
                xts.append(xt)
            return xts

        # ---- K^T projection: KT[jb] is [128 (j), T] ----
        kt_tiles = [p_kt.tile([KB, T], BF16, tag="kt", name=f"kt{j}") for j in range(NJB)]
        for tb in range(NTB):
            xts = load_x_tiles(xkT, tb)
            for jb in range(NJB):
                ps = p_pj.tile([KB, TB], F32, tag="pj")
                for kb in range(NKB):
                    nc.tensor.matmul(
                        ps[:],
                        w_k[:, kb, jb * KB : (jb + 1) * KB],
                        xts[kb][:],
                        start=(kb == 0),
                        stop=(kb == NKB - 1),
                    )
                nc.vector.tensor_scalar_add(
                    kt_tiles[jb][:, tb * TB : (tb + 1) * TB],
                    ps[:],
                    b_k[:, jb : jb + 1],
                )

        # ---- V projection (natural layout): V[tk] is [128 (t), 8, DH+1] ----
        v_tiles = [
            p_v.tile([KB, H // 2, DH + 1], BF16, tag="v", name=f"v{j}")
            for j in range(NTK)
        ]
        for t in range(NTK):
            nc.sync.dma_start(v_tiles[t][:, :, DH : DH + 1], ones8[:, :, None])
        for tb in range(NTB):
            xts = load_x_tiles(xvT, tb)
            for ts in range(4):
                ps = p_pj.tile([KB, TB], F32, tag="pj")
                for kb in range(NKB):
                    nc.tensor.matmul(
                        ps[:],
                        xts[kb][:, ts * KB : (ts + 1) * KB],
                        w_v[:, kb, :],
                        start=(kb == 0),
                        stop=(kb == NKB - 1),
                    )
                nc.vector.tensor_copy(
                    v_tiles[tb * 4 + ts][:, :, 0:DH],
                    ps[:].rearrange("p (h d) -> p h d", d=DH),
                )

        # ---- per t-block: Q^T projection, attention, out-projection ----
        for tq in range(NTB):
            # Q^T for this t-block: qt[jb] [128 (j), TB]
            qt_tiles = [p_qt.tile([KB, TB], BF16, tag="qt", name=f"qt{j}") for j in range(NJB)]
            xts = load_x_tiles(xqT, tq)
            for jb in range(NJB):
                ps = p_pj.tile([KB, TB], F32, tag="pj")
                for kb in range(NKB):
                    nc.tensor.matmul(
                        ps[:],
                        w_q[:, kb, jb * KB : (jb + 1) * KB],
                        xts[kb][:],
                        start=(kb == 0),
                        stop=(kb == NKB - 1),
                    )
                nc.vector.tensor_scalar_add(
                    qt_tiles[jb][:], ps[:], b_q[:, jb : jb + 1]
                )

            ot_tiles = [p_ot.tile([KB, TB], BF16, tag="ot", name=f"ot{j}") for j in range(NJB)]
            for jp in range(NJB):  # head pair (2*jp, 2*jp+1)
                # each head of the pair accumulates into its own psum tile;
                # row DH carries the softmax denominator via the ones column
                # interleaved in V.
                avs = [p_av.tile([DH + 1, TB], F32, tag="av", name=f"av{i}") for i in range(2)]
                for tk in range(NTK):
                    sc = p_sc.tile([KB, 2 * TB], F32, tag="sc")
                    # scores: S^T[tk-block, tq] for both heads of the pair.
                    # The two matmuls hit row groups 0 and 64 and run
                    # concurrently in the PE array.
                    for i in range(2):
                        nc.tensor.matmul(
                            sc[:, i * TB : (i + 1) * TB],
                            kt_tiles[jp][i * DH : (i + 1) * DH, tk * KB : (tk + 1) * KB],
                            qt_tiles[jp][i * DH : (i + 1) * DH, :],
                            start=True,
                            stop=True,
                        )
                    ex = p_ex.tile([KB, 2 * TB], BF16, tag="ex")
                    nc.scalar.activation(
                        ex[:], sc[:], mybir.ActivationFunctionType.Exp, scale=0.125
                    )
                    for i in range(2):
                        nc.tensor.matmul(
                            avs[i][:],
                            v_tiles[tk][:, 2 * jp + i, :],
                            ex[:, i * TB : (i + 1) * TB],
                            start=(tk == 0),
                            stop=(tk == NTK - 1),
                        )
                for i in range(2):
                    # denominator row -> sbuf, DMA-broadcast across the 64
                    # head-dim partitions, reciprocal at base 0 (custom-DVE
                    # ops misbehave on nonzero partition base), then scale
                    dsb = p_rc.tile([DH + 1, TB], F32, tag="dsb")
                    nc.vector.tensor_copy(dsb[DH : DH + 1, :], avs[i][DH : DH + 1, :])
                    bc = p_rc.tile([DH, TB], F32, tag="bc")
                    nc.sync.dma_start(
                        bc[:],
                        dsb[DH : DH + 1, None, :].broadcast_to([1, DH, TB]),
                    )
                    rc2 = p_rc.tile([DH, TB], F32, tag="rc2")
                    nc.vector.reciprocal_approx_fast(rc2[:], bc[:])
                    if i == 0:
                        nc.vector.tensor_mul(ot_tiles[jp][0:DH, :], avs[i][0:DH, :], rc2[:])
                    else:
                        # DVE can't shift partitions; stage then DMA into rows 64:128
                        stg = p_rc.tile([DH, TB], BF16, tag="stg")
                        nc.vector.tensor_mul(stg[:], avs[i][0:DH, :], rc2[:])
                        nc.sync.dma_start(ot_tiles[jp][DH : 2 * DH, :], stg[:])

            # out-projection for this t-block
            for nb in range(2):
                for ts in range(4):
                    po = p_pj.tile([KB, TB], F32, tag="pj")
                    for jp in range(NJB):
                        nc.tensor.matmul(
                            po[:],
                            ot_tiles[jp][:, ts * KB : (ts + 1) * KB],
                            w_o[:, jp, nb * TB : (nb + 1) * TB],
                            start=(jp == 0),
                            stop=(jp == NJB - 1),
                        )
                    st = p_st.tile([KB, TB], F32, tag="st")
                    nc.vector.tensor_copy(st[:], po[:])
                    nc.sync.dma_start(
                        partial[
                            tq * TB + ts * KB : tq * TB + (ts + 1) * KB,
                            nb * TB : (nb + 1) * TB,
                        ],
                        st[:],
                    )

    nc.compile()
    return nc


def kernel(**inputs: np.ndarray) -> np.ndarray:
    query = np.asarray(inputs["query"], dtype=np.float32)
    key = np.asarray(inputs["key"], dtype=np.float32)
    value = np.asarray(inputs["value"], dtype=np.float32)
    w_q = np.asarray(inputs["w_q"], dtype=np.float32)
    b_q = np.asarray(inputs["b_q"], dtype=np.float32)
    w_k = np.asarray(inputs["w_k"], dtype=np.float32)
    b_k = np.asarray(inputs["b_k"], dtype=np.float32)
    w_v = np.asarray(inputs["w_v"], dtype=np.float32)
    b_v = np.asarray(inputs["b_v"], dtype=np.float32)
    w_o = np.asarray(inputs["w_o"], dtype=np.float32)
    b_o = np.asarray(inputs["b_o"], dtype=np.float32)

    nc = build_kernel()

    bf = ml_dtypes.bfloat16
    in_maps = []
    for c in range(N_CORES):
        b = c // 2
        hh = c % 2
        sl = slice(hh * HALF, (hh + 1) * HALF)
        in_maps.append(
            {
                "xqT": np.ascontiguousarray(query[b].T.astype(bf)),
                "xkT": np.ascontiguousarray(key[b].T.astype(bf)),
                "xvT": np.ascontiguousarray(value[b].T.astype(bf)),
                "wqT": np.ascontiguousarray(w_q[sl, :].T.astype(bf)),
                "wkT": np.ascontiguousarray(w_k[sl, :].T.astype(bf)),
                "wvT": np.ascontiguousarray(w_v[sl, :].T.astype(bf)),
                "woT": np.ascontiguousarray(w_o[:, sl].T.astype(bf)),
                "bq": np.ascontiguousarray(b_q[sl].reshape(HALF, 1)),
                "bk": np.ascontiguousarray(b_k[sl].reshape(HALF, 1)),
                "ones_in": np.ones((KB, H // 2), dtype=bf),
            }
        )

    res = run_bass_kernel_spmd(nc, in_maps, core_ids=list(range(N_CORES)))

    const_row = (b_v[None, :] @ w_o.T + b_o[None, :]).astype(np.float32)
    out = np.empty((B, T, D), dtype=np.float32)
    for b in range(B):
        out[b] = res.results[2 * b]["partial"] + res.results[2 * b + 1]["partial"]
        out[b] += const_row
    return out


# revision 4
# speedup vs baseline: 1.9586x; 1.3744x over previous
"""Multi-head attention (B=4, T=2048, D=1024, H=16) on 8 TRN2 NeuronCores.

Sharding: core c handles batch b = c//2 and head-half hh = c%2 (8 heads,
512 of the 1024 channel dims). Each core computes its half of the head
outputs and a row-sharded output projection, producing a partial
[T, D] output. Host unshard: out[b] = partial[2b] + partial[2b+1]
+ b_o + b_v @ w_o.T (the value-bias contribution commutes through
attention because softmax rows sum to 1).

v4: all-bf16 matmuls, ACT-paced softmax pipeline.
  PSUM: scores 2x[128,1024] double-buffered, av 2x[65,512], proj 2x[128,512].
  - The av accumulators spill to SBUF with a single copy at chain end so
    the next head-pair's AV chain reuses the psum slot ~0.7us later; the
    normalize chain (broadcast/reciprocal/multiply) runs off-path from SBUF.
  - Attention starts after the first K/V t-block; remaining projection
    chains are the PE filler under exp latency.
  - Q projection of t-block tq+1 is emitted before out-projection of tq
    so the attention pipeline restarts immediately at tq boundaries.
"""

from contextlib import ExitStack

import numpy as np
import ml_dtypes

import concourse.bass as bass
import concourse.mybir as mybir
import concourse.tile as tile
from concourse import bacc
from concourse.bass_utils import run_bass_kernel_spmd

B, T, D = 4, 2048, 1024
H = 16
DH = 64  # head dim
HALF = 512  # channels per core (8 heads)
N_CORES = 8

F32 = mybir.dt.float32
BF16 = mybir.dt.bfloat16

TB = 512  # t-block for moving operands
NTB = T // TB  # 4
KB = 128  # contraction block
NKB = D // KB  # 8
NJB = HALF // KB  # 4 j-blocks of the half
NTK = T // KB  # 16 tk blocks


def build_kernel():
    nc = bacc.Bacc(
        "TRN2", target_bir_lowering=False, debug=False, num_devices=N_CORES
    )
    xqT = nc.dram_tensor("xqT", [D, T], BF16, kind="ExternalInput").ap()
    xkT = nc.dram_tensor("xkT", [D, T], BF16, kind="ExternalInput").ap()
    xvT = nc.dram_tensor("xvT", [D, T], BF16, kind="ExternalInput").ap()
    wqT = nc.dram_tensor("wqT", [D, HALF], BF16, kind="ExternalInput").ap()
    wkT = nc.dram_tensor("wkT", [D, HALF], BF16, kind="ExternalInput").ap()
    wvT = nc.dram_tensor("wvT", [D, HALF], BF16, kind="ExternalInput").ap()
    woT = nc.dram_tensor("woT", [HALF, D], BF16, kind="ExternalInput").ap()
    bq = nc.dram_tensor("bq", [HALF, 1], F32, kind="ExternalInput").ap()
    bk = nc.dram_tensor("bk", [HALF, 1], F32, kind="ExternalInput").ap()
    ones_in = nc.dram_tensor("ones_in", [KB, H // 2], BF16, kind="ExternalInput").ap()
    partial = nc.dram_tensor("partial", [T, D], F32, kind="ExternalOutput").ap()

    with tile.TileContext(nc) as tc, ExitStack() as ctx:
        p_const = ctx.enter_context(tc.tile_pool(name="const", bufs=1))
        p_kt = ctx.enter_context(tc.tile_pool(name="kt", bufs=NJB * NTB))
        p_v = ctx.enter_context(tc.tile_pool(name="v", bufs=NTK))
        p_qt = ctx.enter_context(tc.tile_pool(name="qt", bufs=2 * NJB))
        p_xs = ctx.enter_context(tc.tile_pool(name="xs", bufs=10))
        p_ex = ctx.enter_context(tc.tile_pool(name="ex", bufs=4))
        p_ot = ctx.enter_context(tc.tile_pool(name="ot", bufs=2 * NJB))
        p_as = ctx.enter_context(tc.tile_pool(name="as", bufs=4))
        p_rc = ctx.enter_context(tc.tile_pool(name="rc", bufs=2))
        p_st = ctx.enter_context(tc.tile_pool(name="st", bufs=2))
        # PSUM: scores 2x2 banks + av 2x1 + proj/outproj 2x1 = 8 banks
        p_sc = ctx.enter_context(tc.tile_pool(name="sc", bufs=2, space="PSUM"))
        p_av = ctx.enter_context(tc.tile_pool(name="av", bufs=2, space="PSUM"))
        p_pj = ctx.enter_context(tc.tile_pool(name="pj", bufs=2, space="PSUM"))

        # ---- constants ----
        w_q = p_const.tile([KB, NKB, HALF], BF16, tag="wq")
        nc.sync.dma_start(w_q[:], wqT.rearrange("(kb p) j -> p kb j", p=KB))
        w_k = p_const.tile([KB, NKB, HALF], BF16, tag="wk")
        nc.sync.dma_start(w_k[:], wkT.rearrange("(kb p) j -> p kb j", p=KB))
        w_v = p_const.tile([KB, NKB, HALF], BF16, tag="wv")
        nc.sync.dma_start(w_v[:], wvT.rearrange("(kb p) j -> p kb j", p=KB))
        w_o = p_const.tile([KB, NJB, D], BF16, tag="wo")
        nc.sync.dma_start(w_o[:], woT.rearrange("(jb p) n -> p jb n", p=KB))
        b_q = p_const.tile([KB, NJB], F32, tag="bq")
        nc.sync.dma_start(b_q[:], bq.rearrange("(jb p) one -> p (jb one)", p=KB))
        b_k = p_const.tile([KB, NJB], F32, tag="bk")
        nc.sync.dma_start(b_k[:], bk.rearrange("(jb p) one -> p (jb one)", p=KB))
        ones8 = p_const.tile([KB, H // 2], BF16, tag="ones8")
        nc.sync.dma_start(ones8[:], ones_in[:])

        def load_x_tiles(src, tb):
            """DMA one t-block of an input into 8 resident [128, 512] tiles."""
            xts = []
            for kb in range(NKB):
                xt = p_xs.tile([KB, TB], BF16, tag="xs")
                nc.sync.dma_start(
                    xt[:], src[kb * KB : (kb + 1) * KB, tb * TB : (tb + 1) * TB]
                )
                xts.append(xt)
            return xts

        # kt[jb][tb]: [128 (j), TB] tiles (separate tiles per t-block so
        # attention groups depend only on the t-blocks they read)
        kt_tiles = [
            [p_kt.tile([KB, TB], BF16, tag="kt", name=f"kt{j}_{tb}") for tb in range(NTB)]
            for j in range(NJB)
        ]
        v_tiles = [
            p_v.tile([KB, H // 2, DH + 1], BF16, tag="v", name=f"v{j}")
            for j in range(NTK)
        ]

        def k_proj(tb):
            xts = load_x_tiles(xkT, tb)
            for jb in range(NJB):
                ps = p_pj.tile([KB, TB], F32, tag="pj")
                for kb in range(NKB):
                    nc.tensor.matmul(
                        ps[:],
                        w_k[:, kb, jb * KB : (jb + 1) * KB],
                        xts[kb][:],
                        start=(kb == 0),
                        stop=(kb == NKB - 1),
                    )
                nc.vector.tensor_scalar_add(
                    kt_tiles[jb][tb][:], ps[:], b_k[:, jb : jb + 1]
                )

        def v_proj(tb):
            for ts in range(4):
                nc.sync.dma_start(
                    v_tiles[tb * 4 + ts][:, :, DH : DH + 1], ones8[:, :, None]
                )
            xts = load_x_tiles(xvT, tb)
            for ts in range(4):
                ps = p_pj.tile([KB, TB], F32, tag="pj")
                for kb in range(NKB):
                    nc.tensor.matmul(
                        ps[:],
                        xts[kb][:, ts * KB : (ts + 1) * KB],
                        w_v[:, kb, :],
                        start=(kb == 0),
                        stop=(kb == NKB - 1),
                    )
                nc.vector.tensor_copy(
                    v_tiles[tb * 4 + ts][:, :, 0:DH],
                    ps[:].rearrange("p (h d) -> p h d", d=DH),
                )

        def q_proj(tq):
            qt_tiles = [
                p_qt.tile([KB, TB], BF16, tag="qt", name=f"qt{j}") for j in range(NJB)
            ]
            xts = load_x_tiles(xqT, tq)
            for jb in range(NJB):
                ps = p_pj.tile([KB, TB], F32, tag="pj")
                for kb in range(NKB):
                    nc.tensor.matmul(
                        ps[:],
                        w_q[:, kb, jb * KB : (jb + 1) * KB],
                        xts[kb][:],
                        start=(kb == 0),
                        stop=(kb == NKB - 1),
                    )
                nc.vector.tensor_scalar_add(
                    qt_tiles[jb][:], ps[:], b_q[:, jb : jb + 1]
                )
            return qt_tiles

        def attention(qt_tiles):
            """One t-block of attention; returns the 4 ot pair-tiles."""
            ot_tiles = [
                p_ot.tile([KB, TB], BF16, tag="ot", name=f"ot{j}") for j in range(NJB)
            ]
            for jp in range(NJB):  # head pair (2*jp, 2*jp+1)
                # each head of the pair accumulates into its own psum tile;
                # row DH carries the softmax denominator via the ones column
                # interleaved in V.
                avs = [
                    p_av.tile([DH + 1, TB], F32, tag="av", name=f"av{i}")
                    for i in range(2)
                ]
                for tk in range(NTK):
                    sc = p_sc.tile([KB, 2 * TB], F32, tag="sc")
                    # scores: S^T[tk-block, tq] for both heads of the pair;
                    # the two matmuls hit row groups 0/64 and run concurrently
                    for i in range(2):
                        nc.tensor.matmul(
                            sc[:, i * TB : (i + 1) * TB],
                            kt_tiles[jp][tk // 4][
                                i * DH : (i + 1) * DH,
                                (tk % 4) * KB : (tk % 4 + 1) * KB,
                            ],
                            qt_tiles[jp][i * DH : (i + 1) * DH, :],
                            start=True,
                            stop=True,
                        )
                    ex = p_ex.tile([KB, 2 * TB], BF16, tag="ex")
                    nc.scalar.activation(
                        ex[:], sc[:], mybir.ActivationFunctionType.Exp, scale=0.125
                    )
                    for i in range(2):
                        nc.tensor.matmul(
                            avs[i][:],
                            v_tiles[tk][:, 2 * jp + i, :],
                            ex[:, i * TB : (i + 1) * TB],
                            start=(tk == 0),
                            stop=(tk == NTK - 1),
                        )
                for i in range(2):
                    # spill the accumulator to SBUF immediately (frees the
                    # psum slot for the next head pair), then normalize
                    # off-path: DMA-broadcast the denominator row across the
                    # 64 head-dim partitions, reciprocal, scale.
                    av_s = p_as.tile([DH + 1, TB], F32, tag="as")
                    nc.vector.tensor_copy(av_s[:], avs[i][:])
                    bc = p_rc.tile([DH, TB], F32, tag="bc")
                    nc.sync.dma_start(
                        bc[:],
                        av_s[DH : DH + 1, None, :].broadcast_to([1, DH, TB]),
                    )
                    rc2 = p_rc.tile([DH, TB], F32, tag="rc2")
                    nc.vector.reciprocal_approx_fast(rc2[:], bc[:])
                    if i == 0:
                        nc.vector.tensor_mul(
                            ot_tiles[jp][0:DH, :], av_s[0:DH, :], rc2[:]
                        )
                    else:
                        # DVE can't shift partitions; stage then DMA into rows 64:128
                        stg = p_rc.tile([DH, TB], BF16, tag="stg")
                        nc.vector.tensor_mul(stg[:], av_s[0:DH, :], rc2[:])
                        nc.sync.dma_start(ot_tiles[jp][DH : 2 * DH, :], stg[:])
            return ot_tiles

        def out_proj(tq, ot_tiles):
            for nb in range(2):
                for ts in range(4):
                    po = p_pj.tile([KB, TB], F32, tag="pj")
                    for jp in range(NJB):
                        nc.tensor.matmul(
                            po[:],
                            ot_tiles[jp][:, ts * KB : (ts + 1) * KB],
                            w_o[:, jp, nb * TB : (nb + 1) * TB],
                            start=(jp == 0),
                            stop=(jp == NJB - 1),
                        )
                    st = p_st.tile([KB, TB], F32, tag="st")
                    nc.vector.tensor_copy(st[:], po[:])
                    nc.sync.dma_start(
                        partial[
                            tq * TB + ts * KB : tq * TB + (ts + 1) * KB,
                            nb * TB : (nb + 1) * TB,
                        ],
                        st[:],
                    )

        # ---- emission order: attention starts after the first K/V t-block;
        # later projection chains fill PE idle under exp latency ----
        k_proj(0)
        v_proj(0)
        qt = q_proj(0)
        for tb in range(1, NTB):
            k_proj(tb)
            v_proj(tb)

        prev = None  # (tq, ot_tiles) pending out-projection
        for tq in range(NTB):
            ot = attention(qt)
            if prev is not None:
                out_proj(prev[0], prev[1])
            if tq + 1 < NTB:
                qt = q_proj(tq + 1)
            prev = (tq, ot)
        out_proj(prev[0], prev[1])

    nc.compile()
    return nc


def kernel(**inputs: np.ndarray) -> np.ndarray:
    query = np.asarray(inputs["query"], dtype=np.float32)
    key = np.asarray(inputs["key"], dtype=np.float32)
    value = np.asarray(inputs["value"], dtype=np.float32)
    w_q = np.asarray(inputs["w_q"], dtype=np.float32)
    b_q = np.asarray(inputs["b_q"], dtype=np.float32)
    w_k = np.asarray(inputs["w_k"], dtype=np.float32)
    b_k = np.asarray(inputs["b_k"], dtype=np.float32)
    w_v = np.asarray(inputs["w_v"], dtype=np.float32)
    b_v = np.asarray(inputs["b_v"], dtype=np.float32)
    w_o = np.asarray(inputs["w_o"], dtype=np.float32)
    b_o = np.asarray(inputs["b_o"], dtype=np.float32)

    nc = build_kernel()

    bf = ml_dtypes.bfloat16
    in_maps = []
    for c in range(N_CORES):
        b = c // 2
        hh = c % 2
        sl = slice(hh * HALF, (hh + 1) * HALF)
        in_maps.append(
            {
                "xqT": np.ascontiguousarray(query[b].T.astype(bf)),
                "xkT": np.ascontiguousarray(key[b].T.astype(bf)),
                "xvT": np.ascontiguousarray(value[b].T.astype(bf)),
                "wqT": np.ascontiguousarray(w_q[sl, :].T.astype(bf)),
                "wkT": np.ascontiguousarray(w_k[sl, :].T.astype(bf)),
                "wvT": np.ascontiguousarray(w_v[sl, :].T.astype(bf)),
                "woT": np.ascontiguousarray(w_o[:, sl].T.astype(bf)),
                "bq": np.ascontiguousarray(b_q[sl].reshape(HALF, 1)),
                "bk": np.ascontiguousarray(b_k[sl].reshape(HALF, 1)),
                "ones_in": np.ones((KB, H // 2), dtype=bf),
            }
        )

    res = run_bass_kernel_spmd(nc, in_maps, core_ids=list(range(N_CORES)))

    const_row = (b_v[None, :] @ w_o.T + b_o[None, :]).astype(np.float32)
    out = np.empty((B, T, D), dtype=np.float32)
    for b in range(B):
        out[b] = res.results[2 * b]["partial"] + res.results[2 * b + 1]["partial"]
        out[b] += const_row
    return out


# revision 9
# speedup vs baseline: 1.9999x; 1.0211x over previous
"""Multi-head attention (B=4, T=2048, D=1024, H=16) on 8 TRN2 NeuronCores.

Sharding: core c handles batch b = c//2 and head-half hh = c%2 (8 heads,
512 of the 1024 channel dims). Each core computes its half of the head
outputs and a row-sharded output projection, producing a partial
[T, D] output. Host unshard: out[b] = partial[2b] + partial[2b+1]
+ b_o + b_v @ w_o.T (the value-bias contribution commutes through
attention because softmax rows sum to 1).

v4: all-bf16 matmuls, ACT-paced softmax pipeline.
  PSUM: scores 2x[128,1024] double-buffered, av 2x[65,512], proj 2x[128,512].
  - The av accumulators spill to SBUF with a single copy at chain end so
    the next head-pair's AV chain reuses the psum slot ~0.7us later; the
    normalize chain (broadcast/reciprocal/multiply) runs off-path from SBUF.
  - Attention starts after the first K/V t-block; remaining projection
    chains are the PE filler under exp latency.
  - Q projection of t-block tq+1 is emitted before out-projection of tq
    so the attention pipeline restarts immediately at tq boundaries.
"""

from contextlib import ExitStack

import numpy as np
import ml_dtypes

import concourse.bass as bass
import concourse.mybir as mybir
import concourse.tile as tile
from concourse import bacc
from concourse.bass_utils import run_bass_kernel_spmd

B, T, D = 4, 2048, 1024
H = 16
DH = 64  # head dim
HALF = 512  # channels per core (8 heads)
N_CORES = 8

F32 = mybir.dt.float32
BF16 = mybir.dt.bfloat16

TB = 512  # t-block for moving operands
NTB = T // TB  # 4
KB = 128  # contraction block
NKB = D // KB  # 8
NJB = HALF // KB  # 4 j-blocks of the half
NTK = T // KB  # 16 tk blocks


def build_kernel():
    nc = bacc.Bacc(
        "TRN2", target_bir_lowering=False, debug=False, num_devices=N_CORES
    )
    xqT = nc.dram_tensor("xqT", [D, T], BF16, kind="ExternalInput").ap()
    xkT = nc.dram_tensor("xkT", [D, T], BF16, kind="ExternalInput").ap()
    xvT = nc.dram_tensor("xvT", [D, T], BF16, kind="ExternalInput").ap()
    wqT = nc.dram_tensor("wqT", [D, HALF], BF16, kind="ExternalInput").ap()
    wkT = nc.dram_tensor("wkT", [D, HALF], BF16, kind="ExternalInput").ap()
    wvT = nc.dram_tensor("wvT", [D, HALF], BF16, kind="ExternalInput").ap()
    woT = nc.dram_tensor("woT", [HALF, D], BF16, kind="ExternalInput").ap()
    bq = nc.dram_tensor("bq", [HALF, 1], F32, kind="ExternalInput").ap()
    bk = nc.dram_tensor("bk", [HALF, 1], F32, kind="ExternalInput").ap()
    ones_in = nc.dram_tensor("ones_in", [KB, H // 2], BF16, kind="ExternalInput").ap()
    partial = nc.dram_tensor("partial", [T, D], F32, kind="ExternalOutput").ap()

    with tile.TileContext(nc) as tc, ExitStack() as ctx:
        p_const = ctx.enter_context(tc.tile_pool(name="const", bufs=1))
        p_kt = ctx.enter_context(tc.tile_pool(name="kt", bufs=NJB * NTB))
        p_v = ctx.enter_context(tc.tile_pool(name="v", bufs=NTK))
        p_qt = ctx.enter_context(tc.tile_pool(name="qt", bufs=2 * NJB))
        p_xs = ctx.enter_context(tc.tile_pool(name="xs", bufs=10))
        p_ex = ctx.enter_context(tc.tile_pool(name="ex", bufs=4))
        p_ot = ctx.enter_context(tc.tile_pool(name="ot", bufs=2 * NJB))
        p_as = ctx.enter_context(tc.tile_pool(name="as", bufs=6))
        p_rc = ctx.enter_context(tc.tile_pool(name="rc", bufs=3))
        p_st = ctx.enter_context(tc.tile_pool(name="st", bufs=2))
        # PSUM: scores 2x2 banks + av 2x1 + proj/outproj 2x1 = 8 banks
        p_sc = ctx.enter_context(tc.tile_pool(name="sc", bufs=2, space="PSUM"))
        p_av = ctx.enter_context(tc.tile_pool(name="av", bufs=2, space="PSUM"))
        p_pj = ctx.enter_context(tc.tile_pool(name="pj", bufs=2, space="PSUM"))

        # ---- constants ----
        w_q = p_const.tile([KB, NKB, HALF], BF16, tag="wq")
        nc.sync.dma_start(w_q[:], wqT.rearrange("(kb p) j -> p kb j", p=KB))
        w_k = p_const.tile([KB, NKB, HALF], BF16, tag="wk")
        nc.sync.dma_start(w_k[:], wkT.rearrange("(kb p) j -> p kb j", p=KB))
        w_v = p_const.tile([KB, NKB, HALF], BF16, tag="wv")
        nc.sync.dma_start(w_v[:], wvT.rearrange("(kb p) j -> p kb j", p=KB))
        w_o = p_const.tile([KB, NJB, D], BF16, tag="wo")
        nc.sync.dma_start(w_o[:], woT.rearrange("(jb p) n -> p jb n", p=KB))
        b_q = p_const.tile([KB, NJB], F32, tag="bq")
        nc.sync.dma_start(b_q[:], bq.rearrange("(jb p) one -> p (jb one)", p=KB))
        b_k = p_const.tile([KB, NJB], F32, tag="bk")
        nc.sync.dma_start(b_k[:], bk.rearrange("(jb p) one -> p (jb one)", p=KB))
        ones8 = p_const.tile([KB, H // 2], BF16, tag="ones8")
        nc.sync.dma_start(ones8[:], ones_in[:])

        def load_x_tiles(src, tb):
            """DMA one t-block of an input into 8 resident [128, 512] tiles."""
            xts = []
            for kb in range(NKB):
                xt = p_xs.tile([KB, TB], BF16, tag="xs")
                nc.sync.dma_start(
                    xt[:], src[kb * KB : (kb + 1) * KB, tb * TB : (tb + 1) * TB]
                )
                xts.append(xt)
            return xts

        # kt[jb][tb]: [128 (j), TB] tiles (separate tiles per t-block so
        # attention groups depend only on the t-blocks they read)
        kt_tiles = [
            [p_kt.tile([KB, TB], BF16, tag="kt", name=f"kt{j}_{tb}") for tb in range(NTB)]
            for j in range(NJB)
        ]
        v_tiles = [
            p_v.tile([KB, H // 2, DH + 1], BF16, tag="v", name=f"v{j}")
            for j in range(NTK)
        ]

        def k_proj(tb):
            xts = load_x_tiles(xkT, tb)
            for jb in range(NJB):
                ps = p_pj.tile([KB, TB], F32, tag="pj")
                for kb in range(NKB):
                    nc.tensor.matmul(
                        ps[:],
                        w_k[:, kb, jb * KB : (jb + 1) * KB],
                        xts[kb][:],
                        start=(kb == 0),
                        stop=(kb == NKB - 1),
                    )
                nc.vector.tensor_scalar_add(
                    kt_tiles[jb][tb][:], ps[:], b_k[:, jb : jb + 1]
                )

        def v_proj(tb):
            for ts in range(4):
                nc.sync.dma_start(
                    v_tiles[tb * 4 + ts][:, :, DH : DH + 1], ones8[:, :, None]
                )
            xts = load_x_tiles(xvT, tb)
            for ts in range(4):
                ps = p_pj.tile([KB, TB], F32, tag="pj")
                for kb in range(NKB):
                    nc.tensor.matmul(
                        ps[:],
                        xts[kb][:, ts * KB : (ts + 1) * KB],
                        w_v[:, kb, :],
                        start=(kb == 0),
                        stop=(kb == NKB - 1),
                    )
                nc.vector.tensor_copy(
                    v_tiles[tb * 4 + ts][:, :, 0:DH],
                    ps[:].rearrange("p (h d) -> p h d", d=DH),
                )

        def q_proj(tq):
            qt_tiles = [
                p_qt.tile([KB, TB], BF16, tag="qt", name=f"qt{j}") for j in range(NJB)
            ]
            xts = load_x_tiles(xqT, tq)
            for jb in range(NJB):
                ps = p_pj.tile([KB, TB], F32, tag="pj")
                for kb in range(NKB):
                    nc.tensor.matmul(
                        ps[:],
                        w_q[:, kb, jb * KB : (jb + 1) * KB],
                        xts[kb][:],
                        start=(kb == 0),
                        stop=(kb == NKB - 1),
                    )
                nc.vector.tensor_scalar_add(
                    qt_tiles[jb][:], ps[:], b_q[:, jb : jb + 1]
                )
            return qt_tiles

        def attention(qt_tiles, filler=()):
            """One t-block of attention; returns the 4 ot pair-tiles.

            filler: thunks (e.g. out-projection chains of the previous
            t-block) emitted between head-pair iterations so the PE always
            has ready work queued behind the softmax pipeline.
            """
            filler = list(filler)
            ot_tiles = [
                p_ot.tile([KB, TB], BF16, tag="ot", name=f"ot{j}") for j in range(NJB)
            ]
            for jp in range(NJB):  # head pair (2*jp, 2*jp+1)
                # each head of the pair accumulates into its own psum tile;
                # row DH carries the softmax denominator via the ones column
                # interleaved in V.
                avs = [
                    p_av.tile([DH + 1, TB], F32, tag="av", name=f"av{i}")
                    for i in range(2)
                ]
                for tk in range(NTK):
                    sc = p_sc.tile([KB, 2 * TB], F32, tag="sc")
                    # scores: S^T[tk-block, tq] for both heads of the pair;
                    # the two matmuls hit row groups 0/64 and run concurrently
                    for i in range(2):
                        nc.tensor.matmul(
                            sc[:, i * TB : (i + 1) * TB],
                            kt_tiles[jp][tk // 4][
                                i * DH : (i + 1) * DH,
                                (tk % 4) * KB : (tk % 4 + 1) * KB,
                            ],
                            qt_tiles[jp][i * DH : (i + 1) * DH, :],
                            start=True,
                            stop=True,
                        )
                    ex = p_ex.tile([KB, 2 * TB], BF16, tag="ex")
                    nc.scalar.activation(
                        ex[:], sc[:], mybir.ActivationFunctionType.Exp, scale=0.125
                    )
                    for i in range(2):
                        nc.tensor.matmul(
                            avs[i][:],
                            v_tiles[tk][:, 2 * jp + i, :],
                            ex[:, i * TB : (i + 1) * TB],
                            start=(tk == 0),
                            stop=(tk == NTK - 1),
                        )
                for i in (1, 0):  # odd head first: its chain has an extra DMA
                    # spill the accumulator to SBUF immediately (frees the
                    # psum slot for the next head pair), then normalize
                    # off-path: DMA-broadcast the denominator row across the
                    # 64 head-dim partitions, reciprocal, scale.
                    av_s = p_as.tile([DH + 1, TB], F32, tag="as")
                    nc.vector.tensor_copy(av_s[:], avs[i][:])
                    bc = p_rc.tile([DH, TB], F32, tag="bc")
                    nc.sync.dma_start(
                        bc[:],
                        av_s[DH : DH + 1, None, :].broadcast_to([1, DH, TB]),
                    )
                    rc2 = p_rc.tile([DH, TB], F32, tag="rc2")
                    nc.vector.reciprocal_approx_fast(rc2[:], bc[:])
                    if i == 0:
                        nc.vector.tensor_mul(
                            ot_tiles[jp][0:DH, :], av_s[0:DH, :], rc2[:]
                        )
                    else:
                        # DVE can't shift partitions; stage then DMA into rows 64:128
                        stg = p_rc.tile([DH, TB], BF16, tag="stg")
                        nc.vector.tensor_mul(stg[:], av_s[0:DH, :], rc2[:])
                        nc.sync.dma_start(ot_tiles[jp][DH : 2 * DH, :], stg[:])
                # weave in ~1/4 of the pending filler chains per head pair
                take = len(filler) // (NJB - jp) if jp < NJB - 1 else len(filler)
                for _ in range(take):
                    filler.pop(0)()
            return ot_tiles

        def out_proj_chains(tq, ot_tiles):
            def chain(nb, ts):
                def emit():
                    po = p_pj.tile([KB, TB], F32, tag="pj")
                    for jp in range(NJB):
                        nc.tensor.matmul(
                            po[:],
                            ot_tiles[jp][:, ts * KB : (ts + 1) * KB],
                            w_o[:, jp, nb * TB : (nb + 1) * TB],
                            start=(jp == 0),
                            stop=(jp == NJB - 1),
                        )
                    st = p_st.tile([KB, TB], F32, tag="st")
                    nc.vector.tensor_copy(st[:], po[:])
                    nc.sync.dma_start(
                        partial[
                            tq * TB + ts * KB : tq * TB + (ts + 1) * KB,
                            nb * TB : (nb + 1) * TB,
                        ],
                        st[:],
                    )

                return emit

            return [chain(nb, ts) for nb in range(2) for ts in range(4)]

        # ---- emission order: attention starts after the first K/V t-block;
        # later projection chains fill PE idle under exp latency ----
        k_proj(0)
        v_proj(0)
        qt = q_proj(0)
        for tb in range(1, NTB):
            k_proj(tb)
            v_proj(tb)

        pending = []  # out-projection chains of the previous t-block
        for tq in range(NTB):
            ot = attention(qt, filler=pending)
            if tq + 1 < NTB:
                qt = q_proj(tq + 1)
            pending = out_proj_chains(tq, ot)
        for c in pending:
            c()

    nc.compile()
    return nc


def kernel(**inputs: np.ndarray) -> np.ndarray:
    query = np.asarray(inputs["query"], dtype=np.float32)
    key = np.asarray(inputs["key"], dtype=np.float32)
    value = np.asarray(inputs["value"], dtype=np.float32)
    w_q = np.asarray(inputs["w_q"], dtype=np.float32)
    b_q = np.asarray(inputs["b_q"], dtype=np.float32)
    w_k = np.asarray(inputs["w_k"], dtype=np.float32)
    b_k = np.asarray(inputs["b_k"], dtype=np.float32)
    w_v = np.asarray(inputs["w_v"], dtype=np.float32)
    b_v = np.asarray(inputs["b_v"], dtype=np.float32)
    w_o = np.asarray(inputs["w_o"], dtype=np.float32)
    b_o = np.asarray(inputs["b_o"], dtype=np.float32)

    nc = build_kernel()

    bf = ml_dtypes.bfloat16
    in_maps = []
    for c in range(N_CORES):
        b = c // 2
        hh = c % 2
        sl = slice(hh * HALF, (hh + 1) * HALF)
        in_maps.append(
            {
                "xqT": np.ascontiguousarray(query[b].T.astype(bf)),
                "xkT": np.ascontiguousarray(key[b].T.astype(bf)),
                "xvT": np.ascontiguousarray(value[b].T.astype(bf)),
                "wqT": np.ascontiguousarray(w_q[sl, :].T.astype(bf)),
                "wkT": np.ascontiguousarray(w_k[sl, :].T.astype(bf)),
                "wvT": np.ascontiguousarray(w_v[sl, :].T.astype(bf)),
                "woT": np.ascontiguousarray(w_o[:, sl].T.astype(bf)),
                "bq": np.ascontiguousarray(b_q[sl].reshape(HALF, 1)),
                "bk": np.ascontiguousarray(b_k[sl].reshape(HALF, 1)),
                "ones_in": np.ones((KB, H // 2), dtype=bf),
            }
        )

    res = run_bass_kernel_spmd(nc, in_maps, core_ids=list(range(N_CORES)))

    const_row = (b_v[None, :] @ w_o.T + b_o[None, :]).astype(np.float32)
    out = np.empty((B, T, D), dtype=np.float32)
    for b in range(B):
        out[b] = res.results[2 * b]["partial"] + res.results[2 * b + 1]["partial"]
        out[b] += const_row
    return out


# revision 13
# speedup vs baseline: 2.1718x; 1.0860x over previous
"""Multi-head attention (B=4, T=2048, D=1024, H=16) on 8 TRN2 NeuronCores.

Sharding: core c handles batch b = c//2 and head-half hh = c%2 (8 heads,
512 of the 1024 channel dims). Each core computes its half of the head
outputs and a row-sharded output projection, producing a partial
[T, D] output. Host unshard: out[b] = partial[2b] + partial[2b+1]
+ b_o + b_v @ w_o.T (the value-bias contribution commutes through
attention because softmax rows sum to 1).

v4: all-bf16 matmuls, ACT-paced softmax pipeline.
  PSUM: scores 2x[128,1024] double-buffered, av 2x[65,512], proj 2x[128,512].
  - The av accumulators spill to SBUF with a single copy at chain end so
    the next head-pair's AV chain reuses the psum slot ~0.7us later; the
    normalize chain (broadcast/reciprocal/multiply) runs off-path from SBUF.
  - Attention starts after the first K/V t-block; remaining projection
    chains are the PE filler under exp latency.
  - Q projection of t-block tq+1 is emitted before out-projection of tq
    so the attention pipeline restarts immediately at tq boundaries.
"""

from contextlib import ExitStack

import numpy as np
import ml_dtypes

import concourse.bass as bass
import concourse.mybir as mybir
import concourse.tile as tile
from concourse import bacc
from concourse.bass_utils import run_bass_kernel_spmd

B, T, D = 4, 2048, 1024
H = 16
DH = 64  # head dim
HALF = 512  # channels per core (8 heads)
N_CORES = 8

F32 = mybir.dt.float32
BF16 = mybir.dt.bfloat16

TB = 512  # t-block for moving operands
NTB = T // TB  # 4
KB = 128  # contraction block
NKB = D // KB  # 8
NJB = HALF // KB  # 4 j-blocks of the half
NTK = T // KB  # 16 tk blocks


def build_kernel():
    nc = bacc.Bacc(
        "TRN2", target_bir_lowering=False, debug=False, num_devices=N_CORES
    )
    xqT = nc.dram_tensor("xqT", [D, T], BF16, kind="ExternalInput").ap()
    xkT = nc.dram_tensor("xkT", [D, T], BF16, kind="ExternalInput").ap()
    xvT = nc.dram_tensor("xvT", [D, T], BF16, kind="ExternalInput").ap()
    wqT = nc.dram_tensor("wqT", [D, HALF], BF16, kind="ExternalInput").ap()
    wkT = nc.dram_tensor("wkT", [D, HALF], BF16, kind="ExternalInput").ap()
    wvT = nc.dram_tensor("wvT", [D, HALF], BF16, kind="ExternalInput").ap()
    woT = nc.dram_tensor("woT", [HALF, D], BF16, kind="ExternalInput").ap()
    bq = nc.dram_tensor("bq", [HALF, 1], F32, kind="ExternalInput").ap()
    bk = nc.dram_tensor("bk", [HALF, 1], F32, kind="ExternalInput").ap()
    ones_in = nc.dram_tensor("ones_in", [KB, H // 2], BF16, kind="ExternalInput").ap()
    ones_bc_in = nc.dram_tensor(
        "ones_bc_in", [DH + 1, DH], BF16, kind="ExternalInput"
    ).ap()
    partial = nc.dram_tensor("partial", [T, D], F32, kind="ExternalOutput").ap()

    with tile.TileContext(nc) as tc, ExitStack() as ctx:
        p_const = ctx.enter_context(tc.tile_pool(name="const", bufs=1))
        p_kt = ctx.enter_context(tc.tile_pool(name="kt", bufs=NJB * NTB))
        p_v = ctx.enter_context(tc.tile_pool(name="v", bufs=NTK))
        p_qt = ctx.enter_context(tc.tile_pool(name="qt", bufs=2 * NJB))
        p_xs = ctx.enter_context(tc.tile_pool(name="xs", bufs=10))
        p_ex = ctx.enter_context(tc.tile_pool(name="ex", bufs=4))
        p_ot = ctx.enter_context(tc.tile_pool(name="ot", bufs=2 * NJB))
        p_as = ctx.enter_context(tc.tile_pool(name="as", bufs=6))
        p_rc = ctx.enter_context(tc.tile_pool(name="rc", bufs=3))
        p_st = ctx.enter_context(tc.tile_pool(name="st", bufs=2))
        # PSUM: scores 2x2 banks + av 2x1 + proj/outproj 2x1 = 8 banks
        p_sc = ctx.enter_context(tc.tile_pool(name="sc", bufs=2, space="PSUM"))
        p_av = ctx.enter_context(tc.tile_pool(name="av", bufs=2, space="PSUM"))
        p_pj = ctx.enter_context(tc.tile_pool(name="pj", bufs=2, space="PSUM"))

        # ---- constants ----
        w_q = p_const.tile([KB, NKB, HALF], BF16, tag="wq")
        nc.sync.dma_start(w_q[:], wqT.rearrange("(kb p) j -> p kb j", p=KB))
        w_k = p_const.tile([KB, NKB, HALF], BF16, tag="wk")
        nc.sync.dma_start(w_k[:], wkT.rearrange("(kb p) j -> p kb j", p=KB))
        w_v = p_const.tile([KB, NKB, HALF], BF16, tag="wv")
        nc.sync.dma_start(w_v[:], wvT.rearrange("(kb p) j -> p kb j", p=KB))
        w_o = p_const.tile([KB, NJB, D], BF16, tag="wo")
        nc.sync.dma_start(w_o[:], woT.rearrange("(jb p) n -> p jb n", p=KB))
        b_q = p_const.tile([KB, NJB], F32, tag="bq")
        nc.sync.dma_start(b_q[:], bq.rearrange("(jb p) one -> p (jb one)", p=KB))
        b_k = p_const.tile([KB, NJB], F32, tag="bk")
        nc.sync.dma_start(b_k[:], bk.rearrange("(jb p) one -> p (jb one)", p=KB))
        ones8 = p_const.tile([KB, H // 2], BF16, tag="ones8")
        nc.sync.dma_start(ones8[:], ones_in[:])
        # [1, 64] of ones at partition 64: lhsT of the K=1 broadcast matmul
        ones_bc = p_const.tile([DH + 1, DH], BF16, tag="onesbc")
        nc.sync.dma_start(ones_bc[:], ones_bc_in[:])

        def load_x_tiles(src, tb):
            """DMA one t-block of an input into 8 resident [128, 512] tiles."""
            xts = []
            for kb in range(NKB):
                xt = p_xs.tile([KB, TB], BF16, tag="xs")
                nc.sync.dma_start(
                    xt[:], src[kb * KB : (kb + 1) * KB, tb * TB : (tb + 1) * TB]
                )
                xts.append(xt)
            return xts

        # kt[jb][tb]: [128 (j), TB] tiles (separate tiles per t-block so
        # attention groups depend only on the t-blocks they read)
        kt_tiles = [
            [p_kt.tile([KB, TB], BF16, tag="kt", name=f"kt{j}_{tb}") for tb in range(NTB)]
            for j in range(NJB)
        ]
        v_tiles = [
            p_v.tile([KB, H // 2, DH + 1], BF16, tag="v", name=f"v{j}")
            for j in range(NTK)
        ]

        def k_proj(tb):
            xts = load_x_tiles(xkT, tb)
            for jb in range(NJB):
                ps = p_pj.tile([KB, TB], F32, tag="pj")
                for kb in range(NKB):
                    nc.tensor.matmul(
                        ps[:],
                        w_k[:, kb, jb * KB : (jb + 1) * KB],
                        xts[kb][:],
                        start=(kb == 0),
                        stop=(kb == NKB - 1),
                    )
                nc.vector.tensor_scalar_add(
                    kt_tiles[jb][tb][:], ps[:], b_k[:, jb : jb + 1]
                )

        def v_proj(tb):
            for ts in range(4):
                nc.sync.dma_start(
                    v_tiles[tb * 4 + ts][:, :, DH : DH + 1], ones8[:, :, None]
                )
            xts = load_x_tiles(xvT, tb)
            for ts in range(4):
                ps = p_pj.tile([KB, TB], F32, tag="pj")
                for kb in range(NKB):
                    nc.tensor.matmul(
                        ps[:],
                        xts[kb][:, ts * KB : (ts + 1) * KB],
                        w_v[:, kb, :],
                        start=(kb == 0),
                        stop=(kb == NKB - 1),
                    )
                nc.vector.tensor_copy(
                    v_tiles[tb * 4 + ts][:, :, 0:DH],
                    ps[:].rearrange("p (h d) -> p h d", d=DH),
                )

        def q_proj(tq):
            qt_tiles = [
                p_qt.tile([KB, TB], BF16, tag="qt", name=f"qt{j}") for j in range(NJB)
            ]
            xts = load_x_tiles(xqT, tq)
            for jb in range(NJB):
                ps = p_pj.tile([KB, TB], F32, tag="pj")
                for kb in range(NKB):
                    nc.tensor.matmul(
                        ps[:],
                        w_q[:, kb, jb * KB : (jb + 1) * KB],
                        xts[kb][:],
                        start=(kb == 0),
                        stop=(kb == NKB - 1),
                    )
                nc.vector.tensor_scalar_add(
                    qt_tiles[jb][:], ps[:], b_q[:, jb : jb + 1]
                )
            return qt_tiles

        def attention(qt_tiles, filler=()):
            """One t-block of attention; returns the 4 ot pair-tiles.

            filler: thunks (e.g. out-projection chains of the previous
            t-block) emitted between head-pair iterations so the PE always
            has ready work queued behind the softmax pipeline.
            """
            filler = list(filler)
            ot_tiles = [
                p_ot.tile([KB, TB], BF16, tag="ot", name=f"ot{j}") for j in range(NJB)
            ]
            for jp in range(NJB):  # head pair (2*jp, 2*jp+1)
                # each head of the pair accumulates into its own psum tile;
                # row DH carries the softmax denominator via the ones column
                # interleaved in V.
                avs = [
                    p_av.tile([DH + 1, TB], F32, tag="av", name=f"av{i}")
                    for i in range(2)
                ]
                for tk in range(NTK):
                    sc = p_sc.tile([KB, 2 * TB], F32, tag="sc")
                    # scores: S^T[tk-block, tq] for both heads of the pair;
                    # the two matmuls hit row groups 0/64 and run concurrently
                    for i in range(2):
                        nc.tensor.matmul(
                            sc[:, i * TB : (i + 1) * TB],
                            kt_tiles[jp][tk // 4][
                                i * DH : (i + 1) * DH,
                                (tk % 4) * KB : (tk % 4 + 1) * KB,
                            ],
                            qt_tiles[jp][i * DH : (i + 1) * DH, :],
                            start=True,
                            stop=True,
                        )
                    ex = p_ex.tile([KB, 2 * TB], BF16, tag="ex")
                    nc.scalar.activation(
                        ex[:], sc[:], mybir.ActivationFunctionType.Exp, scale=0.125
                    )
                    for i in range(2):
                        nc.tensor.matmul(
                            avs[i][:],
                            v_tiles[tk][:, 2 * jp + i, :],
                            ex[:, i * TB : (i + 1) * TB],
                            start=(tk == 0),
                            stop=(tk == NTK - 1),
                        )
                # spill both accumulators to SBUF first (all reads of the av
                # psum slots happen up front so the slots recycle quickly and
                # the K=1 broadcast matmuls below can reuse them)
                av_s = []
                d_bf = []
                for i in range(2):
                    a = p_as.tile([DH, TB], F32, tag="as")
                    nc.vector.tensor_copy(a[:], avs[i][0:DH, :])
                    av_s.append(a)
                    # denominator row -> bf16 rhs for the broadcast matmul
                    db = p_as.tile([DH + 1, TB], BF16, tag="db")
                    nc.vector.tensor_copy(db[DH : DH + 1, :], avs[i][DH : DH + 1, :])
                    d_bf.append(db)
                for i in (1, 0):  # odd head first: its chain has an extra DMA
                    # PE broadcasts the denominator across the 64 head-dim
                    # partitions (ones[1,64].T @ d[1,512]), then reciprocal
                    # and scale on DVE - no DMA on the normalize path.
                    dbc = p_av.tile([DH, TB], F32, tag="av", name="dbc")
                    nc.tensor.matmul(
                        dbc[:],
                        ones_bc[DH : DH + 1, :],
                        d_bf[i][DH : DH + 1, :],
                        start=True,
                        stop=True,
                    )
                    rc2 = p_rc.tile([DH, TB], F32, tag="rc2")
                    nc.vector.reciprocal_approx_fast(rc2[:], dbc[:])
                    if i == 0:
                        nc.vector.tensor_mul(
                            ot_tiles[jp][0:DH, :], av_s[i][:], rc2[:]
                        )
                    else:
                        # DVE can't shift partitions; stage then DMA into rows 64:128
                        stg = p_rc.tile([DH, TB], BF16, tag="stg")
                        nc.vector.tensor_mul(stg[:], av_s[i][:], rc2[:])
                        nc.sync.dma_start(ot_tiles[jp][DH : 2 * DH, :], stg[:])
                # weave in ~1/4 of the pending filler chains per head pair
                take = len(filler) // (NJB - jp) if jp < NJB - 1 else len(filler)
                for _ in range(take):
                    filler.pop(0)()
            return ot_tiles

        def out_proj_chains(tq, ot_tiles):
            def chain(nb, ts):
                def emit():
                    po = p_pj.tile([KB, TB], F32, tag="pj")
                    for jp in range(NJB):
                        nc.tensor.matmul(
                            po[:],
                            ot_tiles[jp][:, ts * KB : (ts + 1) * KB],
                            w_o[:, jp, nb * TB : (nb + 1) * TB],
                            start=(jp == 0),
                            stop=(jp == NJB - 1),
                        )
                    st = p_st.tile([KB, TB], F32, tag="st")
                    nc.vector.tensor_copy(st[:], po[:])
                    nc.sync.dma_start(
                        partial[
                            tq * TB + ts * KB : tq * TB + (ts + 1) * KB,
                            nb * TB : (nb + 1) * TB,
                        ],
                        st[:],
                    )

                return emit

            return [chain(nb, ts) for nb in range(2) for ts in range(4)]

        # ---- emission order: attention starts after the first K/V t-block;
        # later projection chains fill PE idle under exp latency ----
        k_proj(0)
        v_proj(0)
        qt = q_proj(0)
        for tb in range(1, NTB):
            k_proj(tb)
            v_proj(tb)

        pending = []  # out-projection chains of the previous t-block
        for tq in range(NTB):
            ot = attention(qt, filler=pending)
            if tq + 1 < NTB:
                qt = q_proj(tq + 1)
            pending = out_proj_chains(tq, ot)
        for c in pending:
            c()

    nc.compile()
    return nc


def kernel(**inputs: np.ndarray) -> np.ndarray:
    query = np.asarray(inputs["query"], dtype=np.float32)
    key = np.asarray(inputs["key"], dtype=np.float32)
    value = np.asarray(inputs["value"], dtype=np.float32)
    w_q = np.asarray(inputs["w_q"], dtype=np.float32)
    b_q = np.asarray(inputs["b_q"], dtype=np.float32)
    w_k = np.asarray(inputs["w_k"], dtype=np.float32)
    b_k = np.asarray(inputs["b_k"], dtype=np.float32)
    w_v = np.asarray(inputs["w_v"], dtype=np.float32)
    b_v = np.asarray(inputs["b_v"], dtype=np.float32)
    w_o = np.asarray(inputs["w_o"], dtype=np.float32)
    b_o = np.asarray(inputs["b_o"], dtype=np.float32)

    nc = build_kernel()

    bf = ml_dtypes.bfloat16
    in_maps = []
    for c in range(N_CORES):
        b = c // 2
        hh = c % 2
        sl = slice(hh * HALF, (hh + 1) * HALF)
        in_maps.append(
            {
                "xqT": np.ascontiguousarray(query[b].T.astype(bf)),
                "xkT": np.ascontiguousarray(key[b].T.astype(bf)),
                "xvT": np.ascontiguousarray(value[b].T.astype(bf)),
                "wqT": np.ascontiguousarray(w_q[sl, :].T.astype(bf)),
                "wkT": np.ascontiguousarray(w_k[sl, :].T.astype(bf)),
                "wvT": np.ascontiguousarray(w_v[sl, :].T.astype(bf)),
                "woT": np.ascontiguousarray(w_o[:, sl].T.astype(bf)),
                "bq": np.ascontiguousarray(b_q[sl].reshape(HALF, 1)),
                "bk": np.ascontiguousarray(b_k[sl].reshape(HALF, 1)),
                "ones_in": np.ones((KB, H // 2), dtype=bf),
                "ones_bc_in": np.ones((DH + 1, DH), dtype=bf),
            }
        )

    res = run_bass_kernel_spmd(nc, in_maps, core_ids=list(range(N_CORES)))

    const_row = (b_v[None, :] @ w_o.T + b_o[None, :]).astype(np.float32)
    out = np.empty((B, T, D), dtype=np.float32)
    for b in range(B):
        out[b] = res.results[2 * b]["partial"] + res.results[2 * b + 1]["partial"]
        out[b] += const_row
    return out
